# revision 1
# baseline (speedup 1.0000x reference)
"""AdderNet (ResNet20-style, L1-distance convs) on 8 TRN2 NeuronCores.

Self-contained: kernel(**inputs) takes the full unsharded inputs and returns
the full [32, 10] float32 output. Data-parallel over the batch (4 images per
core); BatchNorm batch stats are made exact via a per-conv AllReduce of
(sum, sumsq).

Algorithm per adder conv (out = -sum_k |patch - w|):
  - input activations stored fp16, zero-padded, replicated G times across
    partitions: X8[(g, ci), b, H+2, W+2], G = 128//ci
  - DVE: D = |x_shift - w_col| via two 4x tensor_scalar ops
    (subtract, then bitwise_and 0x7FFF on uint16 views = exact fp16 abs)
  - PE: psum[co, bl-chunk] -= sum_ci D via constant block-diag(-1) lhsT
    (M = Cout, zero-padded columns; all co-batches/shifts accumulate)
  - ACT: BN stats accum (Identity/Square + accum_out), then after the
    cross-core AllReduce, evacuate psum -> A_out = relu(psum*r - m*r)
    (+ residual add via DVE scalar_tensor_tensor where needed)
"""

import numpy as np

CORES = 8
# per-conv count of (cb, s) D-passes routed to the ACT engine
ACT_SHARE = {}
for _b in range(3):
    ACT_SHARE[f"l1b{_b}c1"] = 4
    ACT_SHARE[f"l1b{_b}c2"] = 4
ACT_SHARE["l2tc1"] = 6
for _b in range(2):
    ACT_SHARE[f"l2b{_b}c1"] = 12
    ACT_SHARE[f"l2b{_b}c2"] = 12
ACT_SHARE["l2tc2"] = 12
ACT_SHARE["l3tc1"] = 24
for _b in range(2):
    ACT_SHARE[f"l3b{_b}c1"] = 48
    ACT_SHARE[f"l3b{_b}c2"] = 48
ACT_SHARE["l3tc2"] = 48
BL = 4          # local batch per core
EPS = 1e-5
GB = CORES * BL  # global batch


# --------------------------------------------------------------------------
# network schedule, shared by the graph builder and the host weight packer
# --------------------------------------------------------------------------
# lhsT (cst) variants: (ci, G, cout, form). form "abs": blockdiag(-1) only;
# form "max": blockdiag(-2) + an all-(1/G) column block for the sum-x term
# (sum|x-w| = 2*sum max(x,w) - sum_x - sum_w; sum_w cancels in BN).
CST_VARIANTS = [(16, 8, 16, "abs"), (16, 8, 32, "max"), (32, 4, 32, "max"),
                (32, 4, 64, "max"), (64, 2, 64, "max")]


def cst_layout():
    off = {}
    o = 0
    for (ci, g, co, form) in CST_VARIANTS:
        ncb = co // g
        off[(ci, g, co)] = o
        o += ncb * co + (co if form == "max" else 0)
    return off, o


def conv_meta(ci, co, hin, stride, k):
    g = 128 // ci
    ncb = co // g
    hout = hin // stride
    bl = BL * hout * hout
    nchunk = max(1, bl // 512)
    idx = next(i for i, v in enumerate(CST_VARIANTS) if v[:3] == (ci, g, co))
    return dict(ci=ci, co=co, g=g, ncb=ncb, k=k, stride=stride,
                hin=hin, hout=hout, bl=bl, nchunk=nchunk,
                cst=idx, form=CST_VARIANTS[idx][3])


def make_schedule():
    """Adder convs in emission order. Each entry: meta + weight source +
    dataflow roles. 'wsrc' tells the host packer where the weights live."""
    convs = []

    def add(name, wsrc, ci, co, hin, stride, k, **roles):
        m = conv_meta(ci, co, hin, stride, k)
        m.update(name=name, wsrc=wsrc, **roles)
        convs.append(m)

    # layer1: 3 blocks, 16ch, 32x32. buffers X0..X2 rotate.
    rot = [("X0", "X1", "X2"), ("X2", "X0", "X1"), ("X1", "X2", "X0")]
    for b in range(3):
        i, mid, o = rot[b]
        add(f"l1b{b}c1", ("l1_w", 2 * b), 16, 16, 32, 1, 3, inb=i, outb=mid, evac="relu")
        add(f"l1b{b}c2", ("l1_w", 2 * b + 1), 16, 16, 32, 1, 3, inb=mid, outb=o,
            evac="res", idb=i, idkind="pad")
    # layer2 transition: conv1 (s2) + downsample (1x1 s2), merged stats
    add("l2tc1", ("l2_w0",), 16, 32, 32, 2, 3, inb="X0", outb="Y0", evac="relu",
        grp="g2")
    add("l2td", ("l2_down",), 16, 32, 32, 2, 1, inb="X0", outb="ID2", evac="down",
        grp="g2")
    add("l2tc2", ("l2_ws", 0), 32, 32, 16, 1, 3, inb="Y0", outb="Y1", evac="res",
        idb="ID2", idkind="dense")
    rot2 = [("Y1", "Y2", "Y0"), ("Y0", "Y2", "Y1")]
    for b in range(2):
        i, mid, o = rot2[b]
        add(f"l2b{b}c1", ("l2_ws", 1 + 2 * b), 32, 32, 16, 1, 3, inb=i, outb=mid, evac="relu")
        add(f"l2b{b}c2", ("l2_ws", 2 + 2 * b), 32, 32, 16, 1, 3, inb=mid, outb=o,
            evac="res", idb=i, idkind="pad")
    # layer3
    add("l3tc1", ("l3_w0",), 32, 64, 16, 2, 3, inb="Y1", outb="Z0", evac="relu",
        grp="g3")
    add("l3td", ("l3_down",), 32, 64, 16, 2, 1, inb="Y1", outb="ID3", evac="down",
        grp="g3")
    add("l3tc2", ("l3_ws", 0), 64, 64, 8, 1, 3, inb="Z0", outb="Z1", evac="res",
        idb="ID3", idkind="dense")
    rot3 = [("Z1", "Z2", "Z0"), ("Z0", "Z2", "Z1")]
    for b in range(2):
        i, mid, o = rot3[b]
        add(f"l3b{b}c1", ("l3_ws", 1 + 2 * b), 64, 64, 8, 1, 3, inb=i, outb=mid, evac="relu")
        add(f"l3b{b}c2", ("l3_ws", 2 + 2 * b), 64, 64, 8, 1, 3, inb=mid, outb=o,
            evac="res", idb=i, idkind="pad")
    return convs


SCHED = make_schedule()
NWALL = sum(c["ncb"] * c["k"] * c["k"] for c in SCHED)
CST_OFF, NCST = cst_layout()


def chunk_view(t, meta, c, interior=False, group0_rows=None):
    """AP for bl-chunk c of a [rows, B, H(, +2), W(, +2)] tile.
    interior=True: index into padded tile's interior.
    Returns the slice covering 512 (or bl) elements."""
    ho = meta["hout"]
    rows = slice(0, group0_rows) if group0_rows else slice(None)
    if meta["nchunk"] == 1:
        if interior:
            return t[rows, :, 1:ho + 1, 1:ho + 1]
        return t[rows, :, :, :]
    per = 512 // (ho * ho) if ho * ho <= 512 else 0
    if per:  # whole images per chunk (L2: per=2)
        b0 = c * per
        if interior:
            return t[rows, b0:b0 + per, 1:ho + 1, 1:ho + 1]
        return t[rows, b0:b0 + per, :, :]
    # half-image rows (L1: 512 = 16 rows of 32)
    nrow = 512 // ho
    b_i, part = divmod(c, (ho + nrow - 1) // nrow)
    r0 = part * nrow
    if interior:
        return t[rows, b_i, 1 + r0:1 + r0 + nrow, 1:ho + 1]
    return t[rows, b_i, r0:r0 + nrow, :]


def pieces(meta, c):
    """Rank-<=3 sub-views of bl-chunk c: yields (c0, c1, view_fn).
    view_fn(tile, co, interior) returns the matching AP piece."""
    ho = meta["hout"]
    if ho * ho <= 512:
        per = (512 // (ho * ho)) if meta["nchunk"] > 1 else BL
        w = ho * ho
        for j in range(per):
            b = c * per + j

            def vf(t, co, interior, b=b, ho=ho):
                if interior:
                    return t[0:co, b, 1:ho + 1, 1:ho + 1]
                return t[0:co, b, :, :]

            yield j * w, (j + 1) * w, vf
    else:
        nrow = 512 // ho
        b_i, part = divmod(c, ho // nrow)
        r0 = part * nrow

        def vf(t, co, interior, b_i=b_i, r0=r0, ho=ho, nrow=nrow):
            if interior:
                return t[0:co, b_i, 1 + r0:1 + r0 + nrow, 1:ho + 1]
            return t[0:co, b_i, r0:r0 + nrow, :]

        yield 0, 512, vf


# --------------------------------------------------------------------------
# host-side packing
# --------------------------------------------------------------------------
def get_w(inputs, wsrc):
    a = inputs[wsrc[0]]
    if len(wsrc) > 1:
        a = a[wsrc[1]]
    return a  # [co, ci, k, k]


def pack_host(inputs):
    wall = np.zeros((128, NWALL), np.float32)
    col = 0
    for m in SCHED:
        w = get_w(inputs, m["wsrc"])
        ci, g, k = m["ci"], m["g"], m["k"]
        for cb in range(m["ncb"]):
            for s in range(k * k):
                kh, kw = divmod(s, k)
                for gg in range(g):
                    co = cb * g + gg
                    wall[gg * ci:(gg + 1) * ci, col] = w[co, :, kh, kw]
                col += 1
    assert col == NWALL

    cst = np.zeros((128, NCST), np.float16)
    for (ci, g, co, form) in CST_VARIANTS:
        off = CST_OFF[(ci, g, co)]
        val = -1.0 if form == "abs" else -2.0
        for cb in range(co // g):
            for gg in range(g):
                cst[gg * ci:(gg + 1) * ci, off + cb * co + cb * g + gg] = val
        if form == "max":
            cst[:, off + (co // g) * co: off + (co // g) * co + co] = 1.0 / g

    stemw = inputs["conv1_w"].transpose(2, 3, 1, 0).reshape(27, 16).astype(np.float16)
    fcw = (inputs["fc_w"][:, :, 0, 0].T / 64.0).astype(np.float32)  # [64, 10]
    return wall, -wall, cst, stemw, fcw


# --------------------------------------------------------------------------
# graph builder
# --------------------------------------------------------------------------
_CACHE = {}


def build(debug=False, reps=1, sim1=False):
    from concourse import bacc, mybir, tile

    F16, F32, U16 = mybir.dt.float16, mybir.dt.float32, mybir.dt.uint16
    A = mybir.AluOpType
    AF = mybir.ActivationFunctionType

    nc = bacc.Bacc("TRN2", target_bir_lowering=False, debug=False,
                   num_devices=1 if sim1 else CORES)
    xp_d = nc.dram_tensor("xp", [3, BL, 34, 34], F16, kind="ExternalInput")
    wall_d = nc.dram_tensor("wall", [128, NWALL], F32, kind="ExternalInput")
    nwall_d = nc.dram_tensor("nwall", [128, NWALL], F32, kind="ExternalInput")
    cst_d = nc.dram_tensor("cst", [128, NCST], F16, kind="ExternalInput")
    stemw_d = nc.dram_tensor("stemw", [27, 16], F16, kind="ExternalInput")
    fcw_d = nc.dram_tensor("fcw", [64, 10], F32, kind="ExternalInput")
    out_d = nc.dram_tensor("out", [10, BL], F32, kind="ExternalOutput")
    dbg_d = {}
    if debug:
        for m in SCHED:
            shp = ([m["co"], BL, m["hout"] + 2, m["hout"] + 2]
                   if m["evac"] != "down" else [m["co"], BL, m["hout"], m["hout"]])
            dbg_d[m["name"]] = nc.dram_tensor(f'dbg_{m["name"]}', shp,
                                              mybir.dt.float16, kind="ExternalOutput")
        dbg_d["stem"] = nc.dram_tensor("dbg_stem", [16, BL, 34, 34],
                                       mybir.dt.float16, kind="ExternalOutput")
        dbg_d["pooled"] = nc.dram_tensor("dbg_pooled", [64, BL],
                                         F32, kind="ExternalOutput")
        dbg_d["psfc"] = nc.dram_tensor("dbg_psfc", [10, BL],
                                       F32, kind="ExternalOutput")
        dbg_d["gstfc"] = nc.dram_tensor("dbg_gstfc", [10, 2],
                                        F32, kind="ExternalOutput")

    with tile.TileContext(nc) as tc:
        import contextlib
        with contextlib.ExitStack() as ctx:
            pp = ctx.enter_context(tc.tile_pool(name="persist", bufs=1))
            dp = ctx.enter_context(tc.tile_pool(name="dtiles", bufs=4))
            sp = ctx.enter_context(tc.tile_pool(name="small", bufs=6))
            psp = ctx.enter_context(tc.tile_pool(name="psum", bufs=8, space="PSUM"))
            drp = ctx.enter_context(tc.tile_pool(name="dram", bufs=4, space="DRAM"))

            wall = pp.tile([128, NWALL], F32, tag="wall")
            nwall = pp.tile([128, NWALL], F32, tag="nwall")
            cst = pp.tile([128, NCST], F16, tag="cst")
            stemw = pp.tile([27, 16], F16, tag="stemw")
            fcw = pp.tile([64, 10], F32, tag="fcw")
            mask = pp.tile([128, 1], U16, tag="mask")
            epst = pp.tile([64, 1], F32, tag="epst")
            nc.sync.dma_start(wall[:], wall_d[:])
            nc.sync.dma_start(nwall[:], nwall_d[:])
            nc.sync.dma_start(cst[:], cst_d[:])
            nc.sync.dma_start(stemw[:], stemw_d[:])
            nc.sync.dma_start(fcw[:], fcw_d[:])
            nc.vector.memset(mask[:], 0x7FFF)
            nc.vector.memset(epst[:], EPS)

            # activation buffers (persistent, zeroed once => borders stay 0)
            bufs = {}
            for nm in ("X0", "X1", "X2"):
                bufs[nm] = pp.tile([128, BL, 34, 34], F16, name=nm, tag=nm)
            for nm in ("Y0", "Y1", "Y2"):
                bufs[nm] = pp.tile([128, BL, 18, 18], F16, name=nm, tag=nm)
            for nm in ("Z0", "Z1", "Z2"):
                bufs[nm] = pp.tile([128, BL, 10, 10], F16, name=nm, tag=nm)
            bufs["ID2"] = pp.tile([32, BL, 16, 16], F16, name="ID2", tag="ID2")
            bufs["ID3"] = pp.tile([64, BL, 8, 8], F16, name="ID3", tag="ID3")
            for nm in ("X0", "X1", "X2", "Y0", "Y1", "Y2", "Z0", "Z1", "Z2"):
                nc.vector.memset(bufs[nm][:], 0.0)

            # ---------------- BN helper ----------------
            def bn_finish(gstats_slices, n_elems_list):
                """gstats_slices: list of (ap_S1S2 [co, 2],) after allreduce.
                Returns list of (scale_r, bias_nmr) tile pairs."""
                outs = []
                for (gs, n) in zip(gstats_slices, n_elems_list):
                    co = gs.shape[0]
                    mvec = sp.tile([co, 4], F32, tag="bnm", name="bnm")
                    # m = S1/n
                    nc.vector.tensor_scalar(mvec[:, 0:1], gs[:, 0:1], 1.0 / n, None, A.mult)
                    # msq = m*m
                    nc.vector.tensor_tensor(mvec[:, 1:2], mvec[:, 0:1], mvec[:, 0:1], A.mult)
                    # v = S2/n - msq  (biased var)
                    nc.vector.tensor_scalar(mvec[:, 2:3], gs[:, 1:2], 1.0 / n,
                                            mvec[:, 1:2], A.mult, A.subtract)
                    # s = sqrt(v + eps)
                    nc.scalar.activation(mvec[:, 3:4], mvec[:, 2:3], AF.Sqrt,
                                         bias=epst[0:co, 0:1])
                    rr = sp.tile([co, 2], F32, tag="bnr", name="bnr")
                    nc.vector.reciprocal(rr[:, 0:1], mvec[:, 3:4])
                    # nmr = -m * r
                    nc.vector.tensor_scalar(rr[:, 1:2], mvec[:, 0:1], -1.0,
                                            rr[:, 0:1], A.mult, A.mult)
                    outs.append(rr)
                return outs

            def allreduce(stats_tile, rows, nch):
                """stats_tile [rows, 2, nch] -> allreduced SBUF tile [rows, 2, nch]."""
                sin = drp.tile([rows, 2 * nch], F32, tag="ari", name="ari")
                sout = drp.tile([rows, 2 * nch], F32, tag="aro", name="aro")
                nc.sync.dma_start(sin[:], stats_tile[:, :, :])
                if sim1:
                    nc.gpsimd.dma_start(sout[:], sin[:])
                else:
                    nc.gpsimd.collective_compute(
                        "AllReduce", A.add,
                        replica_groups=[list(range(CORES))],
                        ins=[sin.opt()], outs=[sout.opt()],
                    )
                gst = sp.tile([rows, 2, nch], F32, tag="gst", name="gst")
                nc.sync.dma_start(gst[:, :, :], sout[:])
                return gst

            def psum_stats(psums, meta, stats, row0):
                """ACT accum over each chunk into stats[row0:row0+co, 0:2, :]."""
                co, nch = meta["co"], meta["nchunk"]
                for c in range(nch):
                    junk = dp.tile([co, 512], F16, tag="junk", name="junk")
                    jk = junk[:, 0:psums[c].shape[-1]]
                    nc.vector.tensor_scalar(jk, psums[c][:], 0.0, None, A.add,
                                            A.add,
                                            accum_out=stats[row0:row0 + co, 0, c:c + 1])
                    nc.scalar.activation(jk, psums[c][:], AF.Square,
                                         accum_out=stats[row0:row0 + co, 1, c:c + 1])

            def chunk_sum(gst_ap, co):
                """gst_ap [co, 2, nch] AP -> [co, 2] tile by inner reduce."""
                red = sp.tile([co, 2], F32, tag="red", name="red")
                nc.vector.tensor_reduce(red[:], gst_ap, mybir.AxisListType.X, A.add)
                return red

            # ---------------- adder conv core ----------------
            wall_col = [0]

            def adder_conv(meta):
                """Emit D-production + PE reduction. Returns psum tile list."""
                ci, co, g, ncb, k = meta["ci"], meta["co"], meta["g"], meta["ncb"], meta["k"]
                hin, hout, stride, nch = meta["hin"], meta["hout"], meta["stride"], meta["nchunk"]
                xin = bufs[meta["inb"]]
                cvar = CST_VARIANTS[meta["cst"]]
                coff = CST_OFF[cvar[:3]]
                npart = 512 if nch > 1 else meta["bl"]
                psums = [psp.tile([co, npart], F32, tag="ps", name=f"ps_{meta['name']}_{c}")
                         for c in range(nch)]
                first, last = (0, 0), (ncb - 1, k * k - 1)
                ones_off = coff + ncb * co

                def xview(kh, kw, c=None):
                    if stride == 1:
                        v = xin[:, :, kh:kh + hout, kw:kw + hout]
                        if c is None or nch == 1:
                            return v
                        if hout * hout <= 512:  # whole images per chunk
                            per = 512 // (hout * hout)
                            return xin[:, c * per:(c + 1) * per,
                                       kh:kh + hout, kw:kw + hout]
                        nrow = 512 // hout
                        b_i, part = divmod(c, hout // nrow)
                        r0 = part * nrow
                        return xin[:, b_i, kh + r0:kh + r0 + nrow, kw:kw + hout]
                    if k == 1:  # 1x1 stride-2: interior start
                        v = xin[:, :, 1:1 + 2 * hout:2, 1:1 + 2 * hout:2]
                    else:
                        v = xin[:, :, kh:kh + 2 * hout:2, kw:kw + 2 * hout:2]
                    if c is None or nch == 1:
                        return v
                    per = 512 // (hout * hout)
                    if k == 1:
                        return xin[:, c * per:(c + 1) * per,
                                   1:1 + 2 * hout:2, 1:1 + 2 * hout:2]
                    return xin[:, c * per:(c + 1) * per,
                               kh:kh + 2 * hout:2, kw:kw + 2 * hout:2]

                for cb in range(ncb):
                    for s in range(k * k):
                        kh, kw = divmod(s, k)
                        col = wall_col[0]
                        wall_col[0] += 1
                        d = dp.tile([128, BL, hout, hout], F16,
                                    tag=f"d{hout}", name="d")
                        n_act = ACT_SHARE.get(meta["name"], 0)
                        on_act = (cb * k * k + s) % max(1, (ncb * k * k) // max(1, n_act)) == 0 if n_act else False
                        if meta["form"] == "max":
                            if on_act:
                                # relu-form: same -2 lhsT + sum-x columns;
                                # sum|x-w| = 2*sum relu(x-w) + sum_x + const
                                nc.scalar.activation(d[:], xview(kh, kw), AF.Relu,
                                                     bias=nwall[:, col:col + 1])
                            else:
                                nc.vector.tensor_scalar(d[:], xview(kh, kw),
                                                        wall[:, col:col + 1],
                                                        None, A.max)
                        else:
                            if on_act:
                                nc.scalar.activation(d[:], xview(kh, kw), AF.Abs,
                                                     bias=nwall[:, col:col + 1])
                            else:
                                nc.vector.tensor_scalar(d[:], xview(kh, kw),
                                                        wall[:, col:col + 1],
                                                        None, A.subtract)
                                nc.vector.tensor_scalar(d[:].bitcast(U16),
                                                        d[:].bitcast(U16),
                                                        mask[:], None, A.bitwise_and)
                        for c in range(nch):
                            rhs = chunk_view(d, meta, c)
                            nc.tensor.matmul(
                                psums[c][:, :],
                                cst[:, coff + cb * co:coff + (cb + 1) * co],
                                rhs,
                                start=((cb, s) == first), stop=((cb, s) == last),
                            )
                        if meta["form"] == "max" and cb == 0:
                            # sum-x correction: psum += (1/G) * ones.T @ x_shift
                            for c in range(nch):
                                nc.tensor.matmul(
                                    psums[c][:, :],
                                    cst[:, ones_off:ones_off + co],
                                    xview(kh, kw, c),
                                    start=False, stop=False,
                                )
                return psums

            def evacuate(meta, psums, rr):
                """psum -> A_out (+ replicate). rr = [co, 2] (r, -m*r) tile."""
                co, nch = meta["co"], meta["nchunk"]
                xout = bufs[meta["outb"]]
                kind = meta["evac"]
                for c in range(nch):
                    if kind == "res":
                        idt = bufs[meta["idb"]]
                        t = dp.tile([co, psums[c].shape[-1]], F16, tag="tres",
                                    name="tres")
                        for c0, c1, vf in pieces(meta, c):
                            idv = vf(idt, co, meta["idkind"] == "pad")
                            nc.vector.scalar_tensor_tensor(
                                t[:, c0:c1], psums[c][:, c0:c1], rr[:, 0:1],
                                idv, A.mult, A.add)
                    for c0, c1, vf in pieces(meta, c):
                        if kind == "relu":
                            nc.scalar.activation(vf(xout, co, True),
                                                 psums[c][:, c0:c1], AF.Relu,
                                                 bias=rr[:, 1:2], scale=rr[:, 0:1])
                        elif kind == "down":
                            nc.scalar.activation(vf(xout, co, False),
                                                 psums[c][:, c0:c1], AF.Identity,
                                                 bias=rr[:, 1:2], scale=rr[:, 0:1])
                        else:
                            nc.scalar.activation(vf(xout, co, True), t[:, c0:c1],
                                                 AF.Relu, bias=rr[:, 1:2])
                if kind != "down":
                    G_out = 128 // co  # replication count for the output buffer
                    for g2 in range(1, G_out):
                        nc.sync.dma_start(xout[g2 * co:(g2 + 1) * co], xout[0:co])
                if debug and meta["name"] in dbg_d:
                    nc.sync.dma_start(dbg_d[meta["name"]][:], xout[0:co])

            for _rep in range(reps):
                wall_col[0] = 0
                # ---------------- stem ----------------
                with nc.named_scope("stem"):
                    pt = pp.tile([27, BL, 32, 32], F16, tag="pt")
                    for s in range(9):
                        kh, kw = divmod(s, 3)
                        nc.sync.dma_start(pt[3 * s:3 * s + 3], xp_d[:, :, kh:kh + 32, kw:kw + 32])
                    m_stem = conv_meta(16, 16, 32, 1, 3)  # for chunking geometry only
                    ps_stem = [psp.tile([16, 512], F32, tag="ps", name=f"ps_stem{c}")
                               for c in range(8)]
                    for c in range(8):
                        rhs = chunk_view(pt, m_stem, c)
                        nc.tensor.matmul(ps_stem[c][:, :], stemw[:], rhs, start=True, stop=True)
                    stats = sp.tile([16, 2, 8], F32, tag="st", name="st_stem")
                    psum_stats(ps_stem, m_stem, stats, 0)
                    gst = allreduce(stats, 16, 8)
                    (rr,) = bn_finish([chunk_sum(gst[:, :, :], 16)], [GB * 1024])
                    m_stem.update(outb="X0", evac="relu", name="stem")
                    evacuate(m_stem, ps_stem, rr)

                # ---------------- adder conv layers ----------------
                i = 0
                while i < len(SCHED):
                    meta = SCHED[i]
                    if meta.get("grp"):  # merged pair (tc1 + td)
                        meta2 = SCHED[i + 1]
                        with nc.named_scope(meta["name"]):
                            ps1 = adder_conv(meta)
                        with nc.named_scope(meta2["name"]):
                            ps2 = adder_conv(meta2)
                            co1, co2 = meta["co"], meta2["co"]
                            nch = meta["nchunk"]
                            assert nch == meta2["nchunk"]
                            stats = sp.tile([co1 + co2, 2, nch],
                                            F32, tag="st", name=f"st_{meta['name']}")
                            psum_stats(ps1, meta, stats, 0)
                            psum_stats(ps2, meta2, stats, co1)
                            gst = allreduce(stats, co1 + co2, nch)
                            n = GB * meta["hout"] * meta["hout"]
                            rr1, rr2 = bn_finish(
                                [chunk_sum(gst[0:co1, :, :], co1),
                                 chunk_sum(gst[co1:co1 + co2, :, :], co2)],
                                [n, n])
                            evacuate(meta, ps1, rr1)
                            evacuate(meta2, ps2, rr2)
                        i += 2
                    else:
                        with nc.named_scope(meta["name"]):
                            ps = adder_conv(meta)
                            co, nch = meta["co"], meta["nchunk"]
                            stats = sp.tile([co, 2, nch], F32, tag="st", name=f"st_{meta['name']}")
                            psum_stats(ps, meta, stats, 0)
                            gst = allreduce(stats, co, nch)
                            n = GB * meta["hout"] * meta["hout"]
                            (rr,) = bn_finish([chunk_sum(gst[:, :, :], co)], [n])
                            evacuate(meta, ps, rr)
                        i += 1

                # ---------------- avgpool + fc + final bn ----------------
                with nc.named_scope("fc"):
                    zf = bufs[SCHED[-1]["outb"]]
                    pooled = sp.tile([64, BL], F32, tag="pool", name="pooled")
                    junkp = dp.tile([64, 64], F16, tag="junk", name="junkp")
                    for b in range(BL):
                        nc.scalar.activation(junkp[:], zf[0:64, b, 1:9, 1:9],
                                             AF.Identity,
                                             accum_out=pooled[:, b:b + 1])
                    if debug:
                        nc.sync.dma_start(dbg_d["pooled"][:], pooled[:])
                    ps_fc = psp.tile([10, BL], F32, tag="ps", name="ps_fc")
                    nc.tensor.matmul(ps_fc[:, :], fcw[:], pooled[:], start=True, stop=True)
                    stats = sp.tile([10, 2, 1], F32, tag="st", name="st_fc")
                    junk = dp.tile([10, BL], F16, tag="junk", name="junk_fc")
                    nc.scalar.activation(junk[:], ps_fc[:], AF.Identity,
                                         accum_out=stats[:, 0, 0:1])
                    nc.scalar.activation(junk[:], ps_fc[:], AF.Square,
                                         accum_out=stats[:, 1, 0:1])
                    gst = allreduce(stats, 10, 1)
                    if debug:
                        psfc_sb = sp.tile([10, BL], F32, tag="psfcsb", name="psfc_sb")
                        nc.scalar.copy(psfc_sb[:], ps_fc[:])
                        nc.sync.dma_start(dbg_d["psfc"][:], psfc_sb[:])
                        nc.sync.dma_start(dbg_d["gstfc"][:], gst[:, :, 0])
                    (rr,) = bn_finish([chunk_sum(gst[:, :, :], 10)], [GB])
                    osb = sp.tile([10, BL], F32, tag="osb", name="osb")
                    nc.scalar.activation(osb[:], ps_fc[:], AF.Identity,
                                         bias=rr[:, 1:2], scale=rr[:, 0:1])
                    nc.sync.dma_start(out_d[:], osb[:])

    nc.compile()
    return nc


def get_nc(debug=False, reps=1):
    key = f"nc{debug}_{reps}"
    if key not in _CACHE:
        _CACHE[key] = build(debug, reps)
    return _CACHE[key]


# --------------------------------------------------------------------------
# entry point
# --------------------------------------------------------------------------
def kernel(**inputs):
    from concourse.bass_utils import run_bass_kernel_spmd

    x = inputs["x"]  # [32, 3, 32, 32] f32
    wall, nwall, cst, stemw, fcw = pack_host(inputs)
    xpad = np.zeros((CORES, 3, BL, 34, 34), np.float16)
    xs = x.reshape(CORES, BL, 3, 32, 32).transpose(0, 2, 1, 3, 4)
    xpad[:, :, :, 1:33, 1:33] = xs.astype(np.float16)

    nc = get_nc()
    in_maps = [{"xp": xpad[i], "wall": wall, "nwall": nwall, "cst": cst,
                "stemw": stemw, "fcw": fcw} for i in range(CORES)]
    res = run_bass_kernel_spmd(nc, in_maps, list(range(CORES)))
    out = np.concatenate([r["out"].T for r in res.results], axis=0)
    return out.astype(np.float32)



# revision 4
# speedup vs baseline: 10.7762x; 10.7762x over previous
"""AdderNet (ResNet20-style, L1-distance convs) on 8 TRN2 NeuronCores.

Self-contained: kernel(**inputs) takes the full unsharded inputs and returns
the full [32, 10] float32 output. Data-parallel over the batch (4 images per
core); BatchNorm batch stats are made exact via a per-conv AllReduce of
(sum, sumsq).

Algorithm per adder conv (out = -sum_k |patch - w|):
  - input activations stored fp16, zero-padded, replicated G times across
    partitions: X8[(g, ci), b, H+2, W+2], G = 128//ci
  - DVE: D = |x_shift - w_col| via two 4x tensor_scalar ops
    (subtract, then bitwise_and 0x7FFF on uint16 views = exact fp16 abs)
  - PE: psum[co, bl-chunk] -= sum_ci D via constant block-diag(-1) lhsT
    (M = Cout, zero-padded columns; all co-batches/shifts accumulate)
  - ACT: BN stats accum (Identity/Square + accum_out), then after the
    cross-core AllReduce, evacuate psum -> A_out = relu(psum*r - m*r)
    (+ residual add via DVE scalar_tensor_tensor where needed)
"""

import numpy as np

CORES = 8
# per-conv count of (cb, s) D-passes routed to the ACT engine
ACT_SHARE = {}
for _b in range(3):
    ACT_SHARE[f"l1b{_b}c1"] = 4
    ACT_SHARE[f"l1b{_b}c2"] = 4
ACT_SHARE["l2tc1"] = 6
for _b in range(2):
    ACT_SHARE[f"l2b{_b}c1"] = 12
    ACT_SHARE[f"l2b{_b}c2"] = 12
ACT_SHARE["l2tc2"] = 12
ACT_SHARE["l3tc1"] = 24
for _b in range(2):
    ACT_SHARE[f"l3b{_b}c1"] = 48
    ACT_SHARE[f"l3b{_b}c2"] = 48
ACT_SHARE["l3tc2"] = 48
BL = 4          # local batch per core
EPS = 1e-5
GB = CORES * BL  # global batch


# --------------------------------------------------------------------------
# network schedule, shared by the graph builder and the host weight packer
# --------------------------------------------------------------------------
# lhsT (cst) variants: (ci, G, cout, form). form "abs": blockdiag(-1) only;
# form "max": blockdiag(-2) + an all-(1/G) column block for the sum-x term
# (sum|x-w| = 2*sum max(x,w) - sum_x - sum_w; sum_w cancels in BN).
CST_VARIANTS = [(16, 8, 16, "abs"), (16, 8, 32, "max"), (32, 4, 32, "max"),
                (32, 4, 64, "max"), (64, 2, 64, "max")]


def cst_layout():
    off = {}
    o = 0
    for (ci, g, co, form) in CST_VARIANTS:
        ncb = co // g
        off[(ci, g, co)] = o
        o += ncb * co + (co if form == "max" else 0)
    return off, o


def conv_meta(ci, co, hin, stride, k):
    g = 128 // ci
    ncb = co // g
    hout = hin // stride
    bl = BL * hout * hout
    nchunk = max(1, bl // 512)
    idx = next(i for i, v in enumerate(CST_VARIANTS) if v[:3] == (ci, g, co))
    return dict(ci=ci, co=co, g=g, ncb=ncb, k=k, stride=stride,
                hin=hin, hout=hout, bl=bl, nchunk=nchunk,
                cst=idx, form=CST_VARIANTS[idx][3])


def make_schedule():
    """Adder convs in emission order. Each entry: meta + weight source +
    dataflow roles. 'wsrc' tells the host packer where the weights live."""
    convs = []

    def add(name, wsrc, ci, co, hin, stride, k, **roles):
        m = conv_meta(ci, co, hin, stride, k)
        m.update(name=name, wsrc=wsrc, **roles)
        convs.append(m)

    # layer1: 3 blocks, 16ch, 32x32. buffers X0..X2 rotate.
    rot = [("X0", "X1", "X2"), ("X2", "X0", "X1"), ("X1", "X2", "X0")]
    for b in range(3):
        i, mid, o = rot[b]
        add(f"l1b{b}c1", ("l1_w", 2 * b), 16, 16, 32, 1, 3, inb=i, outb=mid, evac="relu")
        add(f"l1b{b}c2", ("l1_w", 2 * b + 1), 16, 16, 32, 1, 3, inb=mid, outb=o,
            evac="res", idb=i, idkind="pad")
    # layer2 transition: conv1 (s2) + downsample (1x1 s2), merged stats
    add("l2tc1", ("l2_w0",), 16, 32, 32, 2, 3, inb="X0", outb="Y0", evac="relu",
        grp="g2")
    add("l2td", ("l2_down",), 16, 32, 32, 2, 1, inb="X0", outb="ID2", evac="down",
        grp="g2")
    add("l2tc2", ("l2_ws", 0), 32, 32, 16, 1, 3, inb="Y0", outb="Y1", evac="res",
        idb="ID2", idkind="dense")
    rot2 = [("Y1", "Y2", "Y0"), ("Y0", "Y2", "Y1")]
    for b in range(2):
        i, mid, o = rot2[b]
        add(f"l2b{b}c1", ("l2_ws", 1 + 2 * b), 32, 32, 16, 1, 3, inb=i, outb=mid, evac="relu")
        add(f"l2b{b}c2", ("l2_ws", 2 + 2 * b), 32, 32, 16, 1, 3, inb=mid, outb=o,
            evac="res", idb=i, idkind="pad")
    # layer3
    add("l3tc1", ("l3_w0",), 32, 64, 16, 2, 3, inb="Y1", outb="Z0", evac="relu",
        grp="g3")
    add("l3td", ("l3_down",), 32, 64, 16, 2, 1, inb="Y1", outb="ID3", evac="down",
        grp="g3")
    add("l3tc2", ("l3_ws", 0), 64, 64, 8, 1, 3, inb="Z0", outb="Z1", evac="res",
        idb="ID3", idkind="dense")
    rot3 = [("Z1", "Z2", "Z0"), ("Z0", "Z2", "Z1")]
    for b in range(2):
        i, mid, o = rot3[b]
        add(f"l3b{b}c1", ("l3_ws", 1 + 2 * b), 64, 64, 8, 1, 3, inb=i, outb=mid, evac="relu")
        add(f"l3b{b}c2", ("l3_ws", 2 + 2 * b), 64, 64, 8, 1, 3, inb=mid, outb=o,
            evac="res", idb=i, idkind="pad")
    return convs


SCHED = make_schedule()
NWALL = sum(c["ncb"] * c["k"] * c["k"] for c in SCHED)
CST_OFF, NCST = cst_layout()


def chunk_view(t, meta, c, interior=False, group0_rows=None):
    """AP for bl-chunk c of a [rows, B, H(, +2), W(, +2)] tile.
    interior=True: index into padded tile's interior.
    Returns the slice covering 512 (or bl) elements."""
    ho = meta["hout"]
    rows = slice(0, group0_rows) if group0_rows else slice(None)
    if meta["nchunk"] == 1:
        if interior:
            return t[rows, :, 1:ho + 1, 1:ho + 1]
        return t[rows, :, :, :]
    per = 512 // (ho * ho) if ho * ho <= 512 else 0
    if per:  # whole images per chunk (L2: per=2)
        b0 = c * per
        if interior:
            return t[rows, b0:b0 + per, 1:ho + 1, 1:ho + 1]
        return t[rows, b0:b0 + per, :, :]
    # half-image rows (L1: 512 = 16 rows of 32)
    nrow = 512 // ho
    b_i, part = divmod(c, (ho + nrow - 1) // nrow)
    r0 = part * nrow
    if interior:
        return t[rows, b_i, 1 + r0:1 + r0 + nrow, 1:ho + 1]
    return t[rows, b_i, r0:r0 + nrow, :]


def pieces(meta, c):
    """Rank-<=3 sub-views of bl-chunk c: yields (c0, c1, view_fn).
    view_fn(tile, co, interior) returns the matching AP piece."""
    ho = meta["hout"]
    if ho * ho <= 512:
        per = (512 // (ho * ho)) if meta["nchunk"] > 1 else BL
        w = ho * ho
        for j in range(per):
            b = c * per + j

            def vf(t, co, interior, b=b, ho=ho):
                if interior:
                    return t[0:co, b, 1:ho + 1, 1:ho + 1]
                return t[0:co, b, :, :]

            yield j * w, (j + 1) * w, vf
    else:
        nrow = 512 // ho
        b_i, part = divmod(c, ho // nrow)
        r0 = part * nrow

        def vf(t, co, interior, b_i=b_i, r0=r0, ho=ho, nrow=nrow):
            if interior:
                return t[0:co, b_i, 1 + r0:1 + r0 + nrow, 1:ho + 1]
            return t[0:co, b_i, r0:r0 + nrow, :]

        yield 0, 512, vf


# --------------------------------------------------------------------------
# host-side packing
# --------------------------------------------------------------------------
def get_w(inputs, wsrc):
    a = inputs[wsrc[0]]
    if len(wsrc) > 1:
        a = a[wsrc[1]]
    return a  # [co, ci, k, k]


def pack_host(inputs):
    wall = np.zeros((128, NWALL), np.float32)
    col = 0
    for m in SCHED:
        w = get_w(inputs, m["wsrc"])
        ci, g, k = m["ci"], m["g"], m["k"]
        for cb in range(m["ncb"]):
            for s in range(k * k):
                kh, kw = divmod(s, k)
                for gg in range(g):
                    co = cb * g + gg
                    wall[gg * ci:(gg + 1) * ci, col] = w[co, :, kh, kw]
                col += 1
    assert col == NWALL

    cst = np.zeros((128, NCST), np.float16)
    for (ci, g, co, form) in CST_VARIANTS:
        off = CST_OFF[(ci, g, co)]
        val = -1.0 if form == "abs" else -2.0
        for cb in range(co // g):
            for gg in range(g):
                cst[gg * ci:(gg + 1) * ci, off + cb * co + cb * g + gg] = val
        if form == "max":
            cst[:, off + (co // g) * co: off + (co // g) * co + co] = 1.0 / g

    stemw = inputs["conv1_w"].transpose(2, 3, 1, 0).reshape(27, 16).astype(np.float16)
    fcw = (inputs["fc_w"][:, :, 0, 0].T / 64.0).astype(np.float32)  # [64, 10]
    return wall, -wall, cst, stemw, fcw


# --------------------------------------------------------------------------
# graph builder
# --------------------------------------------------------------------------
_CACHE = {}


def build(debug=False, reps=1, sim1=False, nocoll=False):
    from concourse import bacc, mybir, tile

    F16, F32, U16 = mybir.dt.float16, mybir.dt.float32, mybir.dt.uint16
    A = mybir.AluOpType
    AF = mybir.ActivationFunctionType

    nc = bacc.Bacc("TRN2", target_bir_lowering=False, debug=False,
                   num_devices=1 if sim1 else CORES)
    xp_d = nc.dram_tensor("xp", [3, BL, 34, 34], F16, kind="ExternalInput")
    wall_d = nc.dram_tensor("wall", [128, NWALL], F32, kind="ExternalInput")
    nwall_d = nc.dram_tensor("nwall", [128, NWALL], F32, kind="ExternalInput")
    cst_d = nc.dram_tensor("cst", [128, NCST], F16, kind="ExternalInput")
    stemw_d = nc.dram_tensor("stemw", [27, 16], F16, kind="ExternalInput")
    fcw_d = nc.dram_tensor("fcw", [64, 10], F32, kind="ExternalInput")
    out_d = nc.dram_tensor("out", [10, BL], F32, kind="ExternalOutput")
    dbg_d = {}
    if debug:
        for m in SCHED:
            shp = ([m["co"], BL, m["hout"] + 2, m["hout"] + 2]
                   if m["evac"] != "down" else [m["co"], BL, m["hout"], m["hout"]])
            dbg_d[m["name"]] = nc.dram_tensor(f'dbg_{m["name"]}', shp,
                                              mybir.dt.float16, kind="ExternalOutput")
        dbg_d["stem"] = nc.dram_tensor("dbg_stem", [16, BL, 34, 34],
                                       mybir.dt.float16, kind="ExternalOutput")
        dbg_d["pooled"] = nc.dram_tensor("dbg_pooled", [64, BL],
                                         F32, kind="ExternalOutput")
        dbg_d["psfc"] = nc.dram_tensor("dbg_psfc", [10, BL],
                                       F32, kind="ExternalOutput")
        dbg_d["gstfc"] = nc.dram_tensor("dbg_gstfc", [10, 2],
                                        F32, kind="ExternalOutput")

    with tile.TileContext(nc) as tc:
        import contextlib
        with contextlib.ExitStack() as ctx:
            pp = ctx.enter_context(tc.tile_pool(name="persist", bufs=1))
            dp = ctx.enter_context(tc.tile_pool(name="dtiles", bufs=4))
            sp = ctx.enter_context(tc.tile_pool(name="small", bufs=6))
            psp = ctx.enter_context(tc.tile_pool(name="psum", bufs=8, space="PSUM"))
            drp = ctx.enter_context(tc.tile_pool(name="dram", bufs=4, space="DRAM"))

            wall = pp.tile([128, NWALL], F32, tag="wall")
            nwall = pp.tile([128, NWALL], F32, tag="nwall")
            cst = pp.tile([128, NCST], F16, tag="cst")
            stemw = pp.tile([27, 16], F16, tag="stemw")
            fcw = pp.tile([64, 10], F32, tag="fcw")
            mask = pp.tile([128, 1], U16, tag="mask")
            epst = pp.tile([64, 1], F32, tag="epst")
            nc.sync.dma_start(wall[:], wall_d[:])
            nc.sync.dma_start(nwall[:], nwall_d[:])
            nc.sync.dma_start(cst[:], cst_d[:])
            nc.sync.dma_start(stemw[:], stemw_d[:])
            nc.sync.dma_start(fcw[:], fcw_d[:])
            nc.vector.memset(mask[:], 0x7FFF)
            nc.vector.memset(epst[:], EPS)

            # activation buffers (persistent, zeroed once => borders stay 0)
            bufs = {}
            for nm in ("X0", "X1", "X2"):
                bufs[nm] = pp.tile([128, BL, 34, 34], F16, name=nm, tag=nm)
            for nm in ("Y0", "Y1", "Y2"):
                bufs[nm] = pp.tile([128, BL, 18, 18], F16, name=nm, tag=nm)
            for nm in ("Z0", "Z1", "Z2"):
                bufs[nm] = pp.tile([128, BL, 10, 10], F16, name=nm, tag=nm)
            bufs["ID2"] = pp.tile([32, BL, 16, 16], F16, name="ID2", tag="ID2")
            bufs["ID3"] = pp.tile([64, BL, 8, 8], F16, name="ID3", tag="ID3")
            for nm in ("X0", "X1", "X2", "Y0", "Y1", "Y2", "Z0", "Z1", "Z2"):
                nc.vector.memset(bufs[nm][:], 0.0)

            # ---------------- BN helper ----------------
            def bn_finish(gstats_slices, n_elems_list):
                """gstats_slices: list of (ap_S1S2 [co, 2],) after allreduce.
                Returns list of (scale_r, bias_nmr) tile pairs."""
                outs = []
                for (gs, n) in zip(gstats_slices, n_elems_list):
                    co = gs.shape[0]
                    mvec = sp.tile([co, 4], F32, tag="bnm", name="bnm")
                    # m = S1/n
                    nc.vector.tensor_scalar(mvec[:, 0:1], gs[:, 0:1], 1.0 / n, None, A.mult)
                    # msq = m*m
                    nc.vector.tensor_tensor(mvec[:, 1:2], mvec[:, 0:1], mvec[:, 0:1], A.mult)
                    # v = S2/n - msq  (biased var)
                    nc.vector.tensor_scalar(mvec[:, 2:3], gs[:, 1:2], 1.0 / n,
                                            mvec[:, 1:2], A.mult, A.subtract)
                    # s = sqrt(v + eps)
                    nc.scalar.activation(mvec[:, 3:4], mvec[:, 2:3], AF.Sqrt,
                                         bias=epst[0:co, 0:1])
                    rr = sp.tile([co, 2], F32, tag="bnr", name="bnr")
                    nc.vector.reciprocal(rr[:, 0:1], mvec[:, 3:4])
                    # nmr = -m * r
                    nc.vector.tensor_scalar(rr[:, 1:2], mvec[:, 0:1], -1.0,
                                            rr[:, 0:1], A.mult, A.mult)
                    outs.append(rr)
                return outs

            def allreduce(stats_tile, rows, nch):
                """stats_tile [rows, 2, nch] -> allreduced SBUF tile [rows, 2, nch]."""
                sin = drp.tile([rows, 2 * nch], F32, tag="ari", name="ari")
                sout = drp.tile([rows, 2 * nch], F32, tag="aro", name="aro")
                nc.sync.dma_start(sin[:], stats_tile[:, :, :])
                if sim1 or nocoll:
                    nc.gpsimd.dma_start(sout[:], sin[:])
                else:
                    nc.gpsimd.collective_compute(
                        "AllReduce", A.add,
                        replica_groups=[list(range(CORES))],
                        ins=[sin.opt()], outs=[sout.opt()],
                    )
                gst = sp.tile([rows, 2, nch], F32, tag="gst", name="gst")
                nc.sync.dma_start(gst[:, :, :], sout[:])
                return gst

            def psum_stats(psums, meta, stats, row0):
                """ACT accum over each chunk into stats[row0:row0+co, 0:2, :]."""
                co, nch = meta["co"], meta["nchunk"]
                for c in range(nch):
                    junk = dp.tile([co, 512], F16, tag="junk", name="junk")
                    jk = junk[:, 0:psums[c].shape[-1]]
                    nc.vector.tensor_scalar(jk, psums[c][:], 0.0, None, A.add,
                                            A.add,
                                            accum_out=stats[row0:row0 + co, 0, c:c + 1])
                    nc.scalar.activation(jk, psums[c][:], AF.Square,
                                         accum_out=stats[row0:row0 + co, 1, c:c + 1])

            def chunk_sum(gst_ap, co):
                """gst_ap [co, 2, nch] AP -> [co, 2] tile by inner reduce."""
                red = sp.tile([co, 2], F32, tag="red", name="red")
                nc.vector.tensor_reduce(red[:], gst_ap, mybir.AxisListType.X, A.add)
                return red

            # ---------------- adder conv core ----------------
            wall_col = [0]

            def adder_conv(meta):
                """Emit D-production + PE reduction. Returns psum tile list."""
                ci, co, g, ncb, k = meta["ci"], meta["co"], meta["g"], meta["ncb"], meta["k"]
                hin, hout, stride, nch = meta["hin"], meta["hout"], meta["stride"], meta["nchunk"]
                xin = bufs[meta["inb"]]
                cvar = CST_VARIANTS[meta["cst"]]
                coff = CST_OFF[cvar[:3]]
                npart = 512 if nch > 1 else meta["bl"]
                psums = [psp.tile([co, npart], F32, tag="ps", name=f"ps_{meta['name']}_{c}")
                         for c in range(nch)]
                first, last = (0, 0), (ncb - 1, k * k - 1)
                ones_off = coff + ncb * co

                def xview(kh, kw, c=None):
                    if stride == 1:
                        v = xin[:, :, kh:kh + hout, kw:kw + hout]
                        if c is None or nch == 1:
                            return v
                        if hout * hout <= 512:  # whole images per chunk
                            per = 512 // (hout * hout)
                            return xin[:, c * per:(c + 1) * per,
                                       kh:kh + hout, kw:kw + hout]
                        nrow = 512 // hout
                        b_i, part = divmod(c, hout // nrow)
                        r0 = part * nrow
                        return xin[:, b_i, kh + r0:kh + r0 + nrow, kw:kw + hout]
                    if k == 1:  # 1x1 stride-2: interior start
                        v = xin[:, :, 1:1 + 2 * hout:2, 1:1 + 2 * hout:2]
                    else:
                        v = xin[:, :, kh:kh + 2 * hout:2, kw:kw + 2 * hout:2]
                    if c is None or nch == 1:
                        return v
                    per = 512 // (hout * hout)
                    if k == 1:
                        return xin[:, c * per:(c + 1) * per,
                                   1:1 + 2 * hout:2, 1:1 + 2 * hout:2]
                    return xin[:, c * per:(c + 1) * per,
                               kh:kh + 2 * hout:2, kw:kw + 2 * hout:2]

                for cb in range(ncb):
                    for s in range(k * k):
                        kh, kw = divmod(s, k)
                        col = wall_col[0]
                        wall_col[0] += 1
                        d = dp.tile([128, BL, hout, hout], F16,
                                    tag=f"d{hout}", name="d")
                        n_act = ACT_SHARE.get(meta["name"], 0)
                        on_act = (cb * k * k + s) % max(1, (ncb * k * k) // max(1, n_act)) == 0 if n_act else False
                        if meta["form"] == "max":
                            if on_act:
                                # relu-form: same -2 lhsT + sum-x columns;
                                # sum|x-w| = 2*sum relu(x-w) + sum_x + const
                                nc.scalar.activation(d[:], xview(kh, kw), AF.Relu,
                                                     bias=nwall[:, col:col + 1])
                            else:
                                nc.vector.tensor_scalar(d[:], xview(kh, kw),
                                                        wall[:, col:col + 1],
                                                        None, A.max)
                        else:
                            if on_act:
                                nc.scalar.activation(d[:], xview(kh, kw), AF.Abs,
                                                     bias=nwall[:, col:col + 1])
                            else:
                                nc.vector.tensor_scalar(d[:], xview(kh, kw),
                                                        wall[:, col:col + 1],
                                                        None, A.subtract)
                                nc.vector.tensor_scalar(d[:].bitcast(U16),
                                                        d[:].bitcast(U16),
                                                        mask[:], None, A.bitwise_and)
                        for c in range(nch):
                            rhs = chunk_view(d, meta, c)
                            nc.tensor.matmul(
                                psums[c][:, :],
                                cst[:, coff + cb * co:coff + (cb + 1) * co],
                                rhs,
                                start=((cb, s) == first), stop=((cb, s) == last),
                            )
                        if meta["form"] == "max" and cb == 0:
                            # sum-x correction: psum += (1/G) * ones.T @ x_shift
                            for c in range(nch):
                                nc.tensor.matmul(
                                    psums[c][:, :],
                                    cst[:, ones_off:ones_off + co],
                                    xview(kh, kw, c),
                                    start=False, stop=False,
                                )
                return psums

            def evacuate(meta, psums, rr):
                """psum -> A_out (+ replicate). rr = [co, 2] (r, -m*r) tile."""
                co, nch = meta["co"], meta["nchunk"]
                xout = bufs[meta["outb"]]
                kind = meta["evac"]
                for c in range(nch):
                    if kind == "res":
                        idt = bufs[meta["idb"]]
                        t = dp.tile([co, psums[c].shape[-1]], F16, tag="tres",
                                    name="tres")
                        for c0, c1, vf in pieces(meta, c):
                            idv = vf(idt, co, meta["idkind"] == "pad")
                            nc.vector.scalar_tensor_tensor(
                                t[:, c0:c1], psums[c][:, c0:c1], rr[:, 0:1],
                                idv, A.mult, A.add)
                    for c0, c1, vf in pieces(meta, c):
                        if kind == "relu":
                            nc.scalar.activation(vf(xout, co, True),
                                                 psums[c][:, c0:c1], AF.Relu,
                                                 bias=rr[:, 1:2], scale=rr[:, 0:1])
                        elif kind == "down":
                            nc.scalar.activation(vf(xout, co, False),
                                                 psums[c][:, c0:c1], AF.Identity,
                                                 bias=rr[:, 1:2], scale=rr[:, 0:1])
                        else:
                            nc.scalar.activation(vf(xout, co, True), t[:, c0:c1],
                                                 AF.Relu, bias=rr[:, 1:2])
                if kind != "down":
                    G_out = 128 // co  # replication count for the output buffer
                    for g2 in range(1, G_out):
                        nc.sync.dma_start(xout[g2 * co:(g2 + 1) * co], xout[0:co])
                if debug and meta["name"] in dbg_d:
                    nc.sync.dma_start(dbg_d[meta["name"]][:], xout[0:co])

            for _rep in range(reps):
                wall_col[0] = 0
                # ---------------- stem ----------------
                with nc.named_scope("stem"):
                    pt = pp.tile([27, BL, 32, 32], F16, tag="pt")
                    for s in range(9):
                        kh, kw = divmod(s, 3)
                        nc.sync.dma_start(pt[3 * s:3 * s + 3], xp_d[:, :, kh:kh + 32, kw:kw + 32])
                    m_stem = conv_meta(16, 16, 32, 1, 3)  # for chunking geometry only
                    ps_stem = [psp.tile([16, 512], F32, tag="ps", name=f"ps_stem{c}")
                               for c in range(8)]
                    for c in range(8):
                        rhs = chunk_view(pt, m_stem, c)
                        nc.tensor.matmul(ps_stem[c][:, :], stemw[:], rhs, start=True, stop=True)
                    stats = sp.tile([16, 2, 8], F32, tag="st", name="st_stem")
                    psum_stats(ps_stem, m_stem, stats, 0)
                    gst = allreduce(stats, 16, 8)
                    (rr,) = bn_finish([chunk_sum(gst[:, :, :], 16)], [GB * 1024])
                    m_stem.update(outb="X0", evac="relu", name="stem")
                    evacuate(m_stem, ps_stem, rr)

                # ---------------- adder conv layers ----------------
                i = 0
                while i < len(SCHED):
                    meta = SCHED[i]
                    if meta.get("grp"):  # merged pair (tc1 + td)
                        meta2 = SCHED[i + 1]
                        with nc.named_scope(meta["name"]):
                            ps1 = adder_conv(meta)
                        with nc.named_scope(meta2["name"]):
                            ps2 = adder_conv(meta2)
                            co1, co2 = meta["co"], meta2["co"]
                            nch = meta["nchunk"]
                            assert nch == meta2["nchunk"]
                            stats = sp.tile([co1 + co2, 2, nch],
                                            F32, tag="st", name=f"st_{meta['name']}")
                            psum_stats(ps1, meta, stats, 0)
                            psum_stats(ps2, meta2, stats, co1)
                            gst = allreduce(stats, co1 + co2, nch)
                            n = GB * meta["hout"] * meta["hout"]
                            rr1, rr2 = bn_finish(
                                [chunk_sum(gst[0:co1, :, :], co1),
                                 chunk_sum(gst[co1:co1 + co2, :, :], co2)],
                                [n, n])
                            evacuate(meta, ps1, rr1)
                            evacuate(meta2, ps2, rr2)
                        i += 2
                    else:
                        with nc.named_scope(meta["name"]):
                            ps = adder_conv(meta)
                            co, nch = meta["co"], meta["nchunk"]
                            stats = sp.tile([co, 2, nch], F32, tag="st", name=f"st_{meta['name']}")
                            psum_stats(ps, meta, stats, 0)
                            gst = allreduce(stats, co, nch)
                            n = GB * meta["hout"] * meta["hout"]
                            (rr,) = bn_finish([chunk_sum(gst[:, :, :], co)], [n])
                            evacuate(meta, ps, rr)
                        i += 1

                # ---------------- avgpool + fc + final bn ----------------
                with nc.named_scope("fc"):
                    zf = bufs[SCHED[-1]["outb"]]
                    pooled = sp.tile([64, BL], F32, tag="pool", name="pooled")
                    junkp = dp.tile([64, 64], F16, tag="junk", name="junkp")
                    for b in range(BL):
                        nc.scalar.activation(junkp[:], zf[0:64, b, 1:9, 1:9],
                                             AF.Identity,
                                             accum_out=pooled[:, b:b + 1])
                    if debug:
                        nc.sync.dma_start(dbg_d["pooled"][:], pooled[:])
                    ps_fc = psp.tile([10, BL], F32, tag="ps", name="ps_fc")
                    nc.tensor.matmul(ps_fc[:, :], fcw[:], pooled[:], start=True, stop=True)
                    stats = sp.tile([10, 2, 1], F32, tag="st", name="st_fc")
                    junk = dp.tile([10, BL], F16, tag="junk", name="junk_fc")
                    nc.scalar.activation(junk[:], ps_fc[:], AF.Identity,
                                         accum_out=stats[:, 0, 0:1])
                    nc.scalar.activation(junk[:], ps_fc[:], AF.Square,
                                         accum_out=stats[:, 1, 0:1])
                    gst = allreduce(stats, 10, 1)
                    if debug:
                        psfc_sb = sp.tile([10, BL], F32, tag="psfcsb", name="psfc_sb")
                        nc.scalar.copy(psfc_sb[:], ps_fc[:])
                        nc.sync.dma_start(dbg_d["psfc"][:], psfc_sb[:])
                        nc.sync.dma_start(dbg_d["gstfc"][:], gst[:, :, 0])
                    (rr,) = bn_finish([chunk_sum(gst[:, :, :], 10)], [GB])
                    osb = sp.tile([10, BL], F32, tag="osb", name="osb")
                    nc.scalar.activation(osb[:], ps_fc[:], AF.Identity,
                                         bias=rr[:, 1:2], scale=rr[:, 0:1])
                    nc.sync.dma_start(out_d[:], osb[:])

    nc.compile()
    return nc


def get_nc(debug=False, reps=1, nocoll=False):
    key = f"nc{debug}_{reps}_{nocoll}"
    if key not in _CACHE:
        _CACHE[key] = build(debug, reps, nocoll=nocoll)
    return _CACHE[key]


# --------------------------------------------------------------------------
# entry point
# --------------------------------------------------------------------------
def kernel(**inputs):
    from concourse.bass_utils import run_bass_kernel_spmd

    x = inputs["x"]  # [32, 3, 32, 32] f32
    wall, nwall, cst, stemw, fcw = pack_host(inputs)
    xpad = np.zeros((CORES, 3, BL, 34, 34), np.float16)
    xs = x.reshape(CORES, BL, 3, 32, 32).transpose(0, 2, 1, 3, 4)
    xpad[:, :, :, 1:33, 1:33] = xs.astype(np.float16)

    nc = get_nc()
    in_maps = [{"xp": xpad[i], "wall": wall, "nwall": nwall, "cst": cst,
                "stemw": stemw, "fcw": fcw} for i in range(CORES)]
    res = run_bass_kernel_spmd(nc, in_maps, list(range(CORES)))
    out = np.concatenate([r["out"].T for r in res.results], axis=0)
    return out.astype(np.float32)

